# revision 36
# baseline (speedup 1.0000x reference)
"""Bahdanau additive attention on 8 TRN2 NeuronCores (data-parallel over batch).

reference math:
  q_proj = query @ W1 + b1                      # [B, U]
  v_proj = values @ W2 + b2                     # [B, T, U]
  score  = tanh(q_proj[:,None,:] + v_proj) @ Wv + bv   # [B, T, 1]
  aw     = softmax(score, axis=1)
  ctx    = sum(aw * values, axis=1)             # [B, D]
  returns (ctx, aw)

Sharding: batch B=32 split 4-per-core across 8 cores; W1/W2/Wv replicated.
Host pre-transposes values -> [b, D, T] and query -> [D, b] so the
contraction dim D lies on SBUF partitions (contiguous DMA, no on-chip
transpose).  bv is dropped: softmax is shift-invariant so it cancels in
both outputs.  The softmax path (exp, Z, attention weights) stays in
fp32/f32r regardless of MM_DT; only the matmul operands use MM_DT.
"""

import numpy as np

B, T, D, U = 32, 2048, 1024, 1024
N_CORES = 8
BPC = B // N_CORES  # batches per core
P = 128
TTILE = 512
NTT = T // TTILE  # 4
NDB = D // P      # 8
NUB = U // P      # 8

# smalls pack layout (columns of a [P, 64] tensor):
#   0:8  b1^T   8:16  b2^T   16:24  Wv^T   24:56  query^T packed [p, db*4+b]
#   56: ones column (for the K=128 partition-sum matmul)
SM_B1, SM_B2, SM_WV, SM_QT, SM_ONE = 0, 8, 16, 24, 56

MM_DT = "bfloat16"  # "bfloat16" | "float32r" | "float32"

_cache = {}


def build_nc(mm_dt_name=MM_DT):
    """Build + compile the single-core Tile program (SPMD across 8 cores)."""
    from contextlib import ExitStack

    import concourse.bacc as bacc
    import concourse.bass as bass
    import concourse.mybir as mybir
    import concourse.tile as tile

    f32 = mybir.dt.float32
    f32r = mybir.dt.float32r
    mmdt = getattr(mybir.dt, mm_dt_name)
    AF = mybir.ActivationFunctionType

    nc = bacc.Bacc("TRN2", target_bir_lowering=False)

    # all large inputs arrive pre-swizzled to exact SBUF images (host-side
    # numpy transposes), so every DMA moves >=4 KB contiguous runs per row
    vimg = nc.declare_dram_parameter(
        "vimg", [BPC, NTT, P, NDB, TTILE], mmdt, isOutput=False
    )
    w1 = nc.declare_dram_parameter("W1", [P, NUB, NDB, P], mmdt, isOutput=False)
    w2 = nc.declare_dram_parameter("W2", [P, NUB, NDB, P], mmdt, isOutput=False)
    smalls_d = nc.declare_dram_parameter("smalls", [P, 64], mmdt, isOutput=False)
    wv32_d = nc.declare_dram_parameter("wv32", [P, NUB], f32, isOutput=False)
    ones_d = nc.declare_dram_parameter("ones", [1, P], f32r, isOutput=False)
    out_ctx = nc.declare_dram_parameter("out_ctx", [BPC, P, NDB], f32, isOutput=True)
    out_attn = nc.declare_dram_parameter("out_attn", [BPC, T], f32, isOutput=True)

    def as_dve(ap):
        # view an MM-typed AP for VectorE use (f32r must be re-typed; bf16 ok)
        return ap.bitcast(f32) if mmdt is f32r else ap

    with ExitStack() as ctx:
        tc = ctx.enter_context(tile.TileContext(nc))
        singles = ctx.enter_context(tc.tile_pool(name="singles", bufs=1))
        nbuf = 6 if mm_dt_name == "bfloat16" else 3
        vpool = ctx.enter_context(tc.tile_pool(name="vpool", bufs=nbuf))
        thpool = ctx.enter_context(tc.tile_pool(name="thpool", bufs=nbuf))
        epool = ctx.enter_context(tc.tile_pool(name="epool", bufs=2))
        opool = ctx.enter_context(tc.tile_pool(name="opool", bufs=2))
        ppv = ctx.enter_context(tc.tile_pool(name="ppv", bufs=3, space="PSUM"))
        pps = ctx.enter_context(tc.tile_pool(name="pps", bufs=2, space="PSUM"))
        ppb = ctx.enter_context(tc.tile_pool(name="ppb", bufs=2, space="PSUM"))
        ppq = ctx.enter_context(tc.tile_pool(name="ppq", bufs=1, space="PSUM"))

        # ---- stage 0 ----
        smalls = singles.tile([P, 64], mmdt)
        wv32 = singles.tile([P, NUB], f32)
        ones_row = singles.tile([1, P], f32r)
        wv_col = lambda ub: smalls[:, SM_WV + ub : SM_WV + ub + 1]
        qt_blk = lambda db: smalls[:, SM_QT + db * BPC : SM_QT + (db + 1) * BPC]

        w1_sb = singles.tile([P, NUB, NDB, P], mmdt)
        w2_sb = singles.tile([P, NUB, NDB, P], mmdt)
        qb_sb = singles.tile([P, NUB, BPC], f32)

        vt_pre = {}

        def load_vt(b, tt):
            vt = vpool.tile([P, NDB, TTILE], mmdt, tag="vt", name=f"vt_{b}_{tt}")
            nc.sync.dma_start(out=vt, in_=vimg[b, tt])
            return vt

        nc.scalar.dma_start(out=w1_sb[:, 0:4], in_=w1[:, 0:4])
        nc.sync.dma_start(out=w2_sb[:, 0:2], in_=w2[:, 0:2])
        vt_pre[(0, 0)] = load_vt(0, 0)
        nc.sync.dma_start(out=smalls, in_=smalls_d[:, :])
        nc.sync.dma_start(out=wv32, in_=wv32_d[:, :])
        nc.sync.dma_start(out=ones_row, in_=ones_d[:, :])
        bsum_sb = singles.tile([P, NUB], f32)
        nc.vector.tensor_add(
            bsum_sb,
            as_dve(smalls[:, SM_B1 : SM_B1 + 8]),
            as_dve(smalls[:, SM_B2 : SM_B2 + 8]),
        )
        nc.scalar.dma_start(out=w1_sb[:, 4:8], in_=w1[:, 4:8])
        nc.sync.dma_start(out=w2_sb[:, 2:8], in_=w2[:, 2:8])
        vt_pre[(0, 1)] = load_vt(0, 1)
        vt_pre[(0, 2)] = load_vt(0, 2)
        vt_pre[(0, 3)] = load_vt(0, 3)

        # q_proj^T + b1 + b2 (tiny; overlaps the W2/values stream)
        for ub in range(NUB):
            pq = ppq.tile([P, BPC], f32, tag="pq")
            for db in range(NDB):
                nc.tensor.matmul(
                    pq,
                    w1_sb[:, ub, db],
                    qt_blk(db),
                    start=(db == 0),
                    stop=(db == NDB - 1),
                )
            nc.vector.tensor_scalar_add(
                out=qb_sb[:, ub, :], in0=pq, scalar1=bsum_sb[:, ub : ub + 1]
            )

        # ---- main loop over batches and t-tiles ----
        # The final batch runs its last t-tiles at half width so the exposed
        # end-of-kernel context chain (DVE mul+reduce) is shorter.
        for b in range(BPC):
            tiles = [(i, 0, TTILE) for i in range(NTT)]
            e_sb = epool.tile([1, T], f32r, tag="e")
            z_sb = epool.tile([1, NTT + 2], f32, tag="z")
            ctx_acc = opool.tile([P, NDB], f32, tag="ctx_acc")
            vt = None
            for ti, (tt, off, tw) in enumerate(tiles):
                t0 = tt * TTILE + off
                tsl = slice(t0, t0 + tw)
                if off == 0:
                    vt = vt_pre.pop((b, tt), None)
                    if vt is None:
                        vt = load_vt(b, tt)
                ps = pps.tile([1, TTILE], f32, tag="ps")
                # final tile: keep the score reduction on TensorE (idle by
                # then) so the exposed tail chain skips the DVE accumulation
                last_tile = b == BPC - 1 and ti == len(tiles) - 1
                if not last_tile:
                    acc_th = thpool.tile([P, TTILE], mmdt, tag="acc_th")
                for ub in range(NUB):
                    pv = ppv.tile([P, TTILE], f32, tag="pv")
                    for db in range(NDB):
                        nc.tensor.matmul(
                            pv[:, :tw],
                            w2_sb[:, ub, db],
                            vt[:, db, off : off + tw],
                            start=(db == 0),
                            stop=(db == NDB - 1),
                        )
                    th = thpool.tile([P, TTILE], mmdt, tag="th")
                    nc.scalar.activation(
                        out=th[:, :tw],
                        in_=pv[:, :tw],
                        func=AF.Tanh,
                        bias=qb_sb[:, ub, b : b + 1],
                    )
                    # fold Wv on VectorE; partition-sum later via one matmul
                    if last_tile:
                        nc.tensor.matmul(
                            ps[:, :tw],
                            wv_col(ub),
                            th[:, :tw],
                            start=(ub == 0),
                            stop=(ub == NUB - 1),
                        )
                    elif ub == 0:
                        nc.vector.tensor_scalar_mul(
                            out=acc_th[:, :tw],
                            in0=th[:, :tw],
                            scalar1=wv32[:, ub : ub + 1],
                        )
                    else:
                        th2 = thpool.tile([P, TTILE], mmdt, tag="th2")
                        nc.vector.tensor_scalar_mul(
                            out=th2[:, :tw],
                            in0=th[:, :tw],
                            scalar1=wv32[:, ub : ub + 1],
                        )
                        nc.vector.tensor_add(
                            acc_th[:, :tw], acc_th[:, :tw], th2[:, :tw]
                        )
                if not last_tile:
                    nc.tensor.matmul(
                        ps[:, :tw],
                        smalls[:, SM_ONE : SM_ONE + 1],
                        acc_th[:, :tw],
                        start=True,
                        stop=True,
                    )
                # exp(score) with fused partial-sum for Z (softmax needs no
                # max-subtraction: |score| <= sum|Wv| ~ 26, safe in fp32)
                nc.scalar.activation(
                    out=e_sb[:, tsl],
                    in_=ps[:, :tw],
                    func=AF.Exp,
                    accum_out=z_sb[:, ti : ti + 1],
                )
                # broadcast e across partitions via K=1 ones-matmul (f32r),
                # bounce to SBUF on ScalarE so the muls run in DVE fast mode
                pb = ppb.tile([P, TTILE], f32, tag="pb")
                nc.tensor.matmul(
                    pb[:, :tw], ones_row, e_sb[:, tsl], start=True, stop=True
                )
                pb_sb = thpool.tile([P, TTILE], mmdt, tag="pb_sb")
                nc.scalar.copy(pb_sb[:, :tw], pb[:, :tw])
                # ctx_acc[p, db] += sum_t vt[p, db, t] * e[t]
                cols = thpool.tile([P, NDB], f32, tag="cols")
                for db in range(NDB):
                    scr = thpool.tile([P, TTILE], mmdt, tag="scr")
                    nc.vector.tensor_mul(
                        scr[:, :tw], as_dve(vt[:, db, off : off + tw]), pb_sb[:, :tw]
                    )
                    nc.vector.reduce_sum(
                        out=cols[:, db : db + 1],
                        in_=scr[:, :tw],
                        axis=mybir.AxisListType.X,
                    )
                if ti == 0:
                    nc.vector.tensor_copy(ctx_acc, cols)
                else:
                    nc.vector.tensor_add(ctx_acc, ctx_acc, cols)
            # ---- per-batch epilogue: normalize ----
            zsum = opool.tile([1, 1], f32, tag="zsum")
            nc.vector.reduce_sum(
                out=zsum, in_=z_sb[:, : len(tiles)], axis=mybir.AxisListType.X
            )
            rz = opool.tile([1, 1], f32, tag="rz")
            nc.vector.reciprocal(out=rz, in_=zsum)
            aw = opool.tile([1, T], f32, tag="aw")
            nc.scalar.activation(
                out=aw, in_=e_sb.bitcast(f32), func=AF.Copy, scale=rz
            )
            nc.sync.dma_start(out=out_attn[b : b + 1, :], in_=aw)
            prz = ppb.tile([P, 1], f32, tag="pb")
            nc.tensor.matmul(prz, ones_row.bitcast(f32), rz, start=True, stop=True)
            rz128 = opool.tile([P, 1], f32, tag="rz128")
            nc.vector.tensor_copy(rz128, prz)
            ctxo = opool.tile([P, NDB], f32, tag="ctxo")
            nc.scalar.activation(
                out=ctxo, in_=ctx_acc, func=AF.Copy, scale=rz128
            )
            nc.sync.dma_start(out=out_ctx[b], in_=ctxo)

    nc.compile()
    return nc


def _np_dt(mm_dt_name):
    if mm_dt_name == "bfloat16":
        import ml_dtypes

        return np.dtype(ml_dtypes.bfloat16)
    return np.float32


def make_in_maps(query, values, W1, b1, W2, b2, Wv, bv, mm_dt_name=MM_DT):
    """Shard + pre-transpose host-side inputs for the 8 cores."""
    del bv  # shift-invariant under softmax; cancels in both outputs
    ndt = _np_dt(mm_dt_name)
    q = np.asarray(query, np.float32)
    # values -> SBUF image [B, NTT, P, NDB, TTILE]: vimg[b,tt,p,db,t] =
    # values[b, tt*TTILE+t, db*P+p]
    vimg = np.ascontiguousarray(
        np.asarray(values, np.float32)
        .reshape(B, NTT, TTILE, NDB, P)
        .transpose(0, 1, 4, 3, 2)
    ).astype(ndt)
    # weights -> SBUF image [P, NUB, NDB, P]: img[p,ub,db,u] = W[db*P+p, ub*P+u]
    def wimg(W):
        return np.ascontiguousarray(
            np.asarray(W, np.float32)
            .reshape(NDB, P, NUB, P)
            .transpose(1, 2, 0, 3)
        ).astype(ndt)

    W1c, W2c = wimg(W1), wimg(W2)
    in_maps = []
    for i in range(N_CORES):
        s = slice(i * BPC, (i + 1) * BPC)
        smalls = np.zeros((P, 64), np.float32)
        smalls[:, SM_ONE] = 1.0
        smalls[:, SM_B1 : SM_B1 + 8] = np.asarray(b1, np.float32).reshape(8, P).T
        smalls[:, SM_B2 : SM_B2 + 8] = np.asarray(b2, np.float32).reshape(8, P).T
        smalls[:, SM_WV : SM_WV + 8] = (
            np.asarray(Wv, np.float32).reshape(8, P, 1)[:, :, 0].T
        )
        # query^T packed: smalls[p, SM_QT + db*4 + b] = query[s][b, db*128+p]
        qs = q[s]  # [4, 1024]
        smalls[:, SM_QT : SM_QT + 32] = (
            qs.reshape(BPC, NDB, P).transpose(2, 1, 0).reshape(P, NDB * BPC)
        )
        in_maps.append(
            {
                "vimg": vimg[s],
                "W1": W1c,
                "W2": W2c,
                "smalls": smalls.astype(ndt),
                "wv32": np.ascontiguousarray(
                    np.asarray(Wv, np.float32).reshape(8, P, 1)[:, :, 0].T
                ),
                "ones": np.ones((1, P), np.float32),
            }
        )
    return in_maps


def kernel(query, values, W1, b1, W2, b2, Wv, bv):
    from concourse.bass_utils import run_bass_kernel_spmd

    if "nc" not in _cache:
        _cache["nc"] = build_nc()
    nc = _cache["nc"]
    in_maps = make_in_maps(query, values, W1, b1, W2, b2, Wv, bv)
    res = run_bass_kernel_spmd(nc, in_maps, core_ids=list(range(N_CORES)))
    ctx = np.concatenate(
        [
            res.results[i]["out_ctx"].transpose(0, 2, 1).reshape(BPC, D)
            for i in range(N_CORES)
        ],
        axis=0,
    )
    aw = np.concatenate([res.results[i]["out_attn"] for i in range(N_CORES)], axis=0)
    return ctx, aw[:, :, None]


# revision 37
# speedup vs baseline: 1.0170x; 1.0170x over previous
"""Bahdanau additive attention on 8 TRN2 NeuronCores (data-parallel over batch).

reference math:
  q_proj = query @ W1 + b1                      # [B, U]
  v_proj = values @ W2 + b2                     # [B, T, U]
  score  = tanh(q_proj[:,None,:] + v_proj) @ Wv + bv   # [B, T, 1]
  aw     = softmax(score, axis=1)
  ctx    = sum(aw * values, axis=1)             # [B, D]
  returns (ctx, aw)

Sharding: batch B=32 split 4-per-core across 8 cores; W1/W2/Wv replicated.
Host pre-transposes values -> [b, D, T] and query -> [D, b] so the
contraction dim D lies on SBUF partitions (contiguous DMA, no on-chip
transpose).  bv is dropped: softmax is shift-invariant so it cancels in
both outputs.  The softmax path (exp, Z, attention weights) stays in
fp32/f32r regardless of MM_DT; only the matmul operands use MM_DT.
"""

import numpy as np

B, T, D, U = 32, 2048, 1024, 1024
N_CORES = 8
BPC = B // N_CORES  # batches per core
P = 128
TTILE = 512
NTT = T // TTILE  # 4
NDB = D // P      # 8
NUB = U // P      # 8

# smalls pack layout (columns of a [P, 64] tensor):
#   0:8  b1^T   8:16  b2^T   16:24  Wv^T   24:56  query^T packed [p, db*4+b]
#   56: ones column (for the K=128 partition-sum matmul)
SM_B1, SM_B2, SM_WV, SM_QT, SM_ONE = 0, 8, 16, 24, 56

MM_DT = "bfloat16"  # "bfloat16" | "float32r" | "float32"

_cache = {}


def build_nc(mm_dt_name=MM_DT):
    """Build + compile the single-core Tile program (SPMD across 8 cores)."""
    from contextlib import ExitStack

    import concourse.bacc as bacc
    import concourse.bass as bass
    import concourse.mybir as mybir
    import concourse.tile as tile

    f32 = mybir.dt.float32
    f32r = mybir.dt.float32r
    mmdt = getattr(mybir.dt, mm_dt_name)
    AF = mybir.ActivationFunctionType

    nc = bacc.Bacc("TRN2", target_bir_lowering=False)

    # all large inputs arrive pre-swizzled to exact SBUF images (host-side
    # numpy transposes), so every DMA moves >=4 KB contiguous runs per row
    vimg = nc.declare_dram_parameter(
        "vimg", [BPC, NTT, P, NDB, TTILE], mmdt, isOutput=False
    )
    w1 = nc.declare_dram_parameter("W1", [P, NUB, NDB, P], mmdt, isOutput=False)
    w2 = nc.declare_dram_parameter("W2", [P, NUB, NDB, P], mmdt, isOutput=False)
    smalls_d = nc.declare_dram_parameter("smalls", [P, 64], mmdt, isOutput=False)
    wv32_d = nc.declare_dram_parameter("wv32", [P, NUB], f32, isOutput=False)
    ones_d = nc.declare_dram_parameter("ones", [1, P], f32r, isOutput=False)
    out_ctx = nc.declare_dram_parameter("out_ctx", [BPC, P, NDB], f32, isOutput=True)
    out_attn = nc.declare_dram_parameter("out_attn", [BPC, T], f32, isOutput=True)

    def as_dve(ap):
        # view an MM-typed AP for VectorE use (f32r must be re-typed; bf16 ok)
        return ap.bitcast(f32) if mmdt is f32r else ap

    with ExitStack() as ctx:
        tc = ctx.enter_context(tile.TileContext(nc))
        singles = ctx.enter_context(tc.tile_pool(name="singles", bufs=1))
        nbuf = 6 if mm_dt_name == "bfloat16" else 3
        vpool = ctx.enter_context(tc.tile_pool(name="vpool", bufs=nbuf))
        thpool = ctx.enter_context(tc.tile_pool(name="thpool", bufs=nbuf))
        epool = ctx.enter_context(tc.tile_pool(name="epool", bufs=2))
        opool = ctx.enter_context(tc.tile_pool(name="opool", bufs=2))
        ppv = ctx.enter_context(tc.tile_pool(name="ppv", bufs=3, space="PSUM"))
        pps = ctx.enter_context(tc.tile_pool(name="pps", bufs=2, space="PSUM"))
        ppb = ctx.enter_context(tc.tile_pool(name="ppb", bufs=2, space="PSUM"))
        ppq = ctx.enter_context(tc.tile_pool(name="ppq", bufs=1, space="PSUM"))

        # ---- stage 0 ----
        smalls = singles.tile([P, 64], mmdt)
        nc.sync.dma_start(out=smalls, in_=smalls_d[:, :])
        wv32 = singles.tile([P, NUB], f32)
        nc.sync.dma_start(out=wv32, in_=wv32_d[:, :])
        ones_row = singles.tile([1, P], f32r)
        nc.sync.dma_start(out=ones_row, in_=ones_d[:, :])
        bsum_sb = singles.tile([P, NUB], f32)
        nc.vector.tensor_add(
            bsum_sb,
            as_dve(smalls[:, SM_B1 : SM_B1 + 8]),
            as_dve(smalls[:, SM_B2 : SM_B2 + 8]),
        )
        wv_col = lambda ub: smalls[:, SM_WV + ub : SM_WV + ub + 1]
        qt_blk = lambda db: smalls[:, SM_QT + db * BPC : SM_QT + (db + 1) * BPC]

        w1_sb = singles.tile([P, NUB, NDB, P], mmdt)
        w2_sb = singles.tile([P, NUB, NDB, P], mmdt)
        qb_sb = singles.tile([P, NUB, BPC], f32)

        vt_pre = {}

        def load_vt(b, tt):
            vt = vpool.tile([P, NDB, TTILE], mmdt, tag="vt", name=f"vt_{b}_{tt}")
            nc.sync.dma_start(out=vt, in_=vimg[b, tt])
            return vt

        nc.scalar.dma_start(out=w1_sb[:, 0:4], in_=w1[:, 0:4])
        nc.sync.dma_start(out=w2_sb[:, 0:2], in_=w2[:, 0:2])
        vt_pre[(0, 0)] = load_vt(0, 0)
        nc.scalar.dma_start(out=w1_sb[:, 4:8], in_=w1[:, 4:8])
        nc.sync.dma_start(out=w2_sb[:, 2:8], in_=w2[:, 2:8])
        vt_pre[(0, 1)] = load_vt(0, 1)
        vt_pre[(0, 2)] = load_vt(0, 2)

        # q_proj^T + b1 + b2 (tiny; overlaps the W2/values stream)
        for ub in range(NUB):
            pq = ppq.tile([P, BPC], f32, tag="pq")
            for db in range(NDB):
                nc.tensor.matmul(
                    pq,
                    w1_sb[:, ub, db],
                    qt_blk(db),
                    start=(db == 0),
                    stop=(db == NDB - 1),
                )
            nc.vector.tensor_scalar_add(
                out=qb_sb[:, ub, :], in0=pq, scalar1=bsum_sb[:, ub : ub + 1]
            )

        # ---- main loop over batches and t-tiles ----
        # The final batch runs its last t-tiles at half width so the exposed
        # end-of-kernel context chain (DVE mul+reduce) is shorter.
        for b in range(BPC):
            tiles = [(i, 0, TTILE) for i in range(NTT)]
            e_sb = epool.tile([1, T], f32r, tag="e")
            z_sb = epool.tile([1, NTT + 2], f32, tag="z")
            ctx_acc = opool.tile([P, NDB], f32, tag="ctx_acc")
            vt = None
            for ti, (tt, off, tw) in enumerate(tiles):
                t0 = tt * TTILE + off
                tsl = slice(t0, t0 + tw)
                if off == 0:
                    vt = vt_pre.pop((b, tt), None)
                    if vt is None:
                        vt = load_vt(b, tt)
                ps = pps.tile([1, TTILE], f32, tag="ps")
                # final tile: keep the score reduction on TensorE (idle by
                # then) so the exposed tail chain skips the DVE accumulation
                last_tile = b == BPC - 1 and ti == len(tiles) - 1
                if not last_tile:
                    acc_th = thpool.tile([P, TTILE], mmdt, tag="acc_th")
                for ub in range(NUB):
                    pv = ppv.tile([P, TTILE], f32, tag="pv")
                    for db in range(NDB):
                        nc.tensor.matmul(
                            pv[:, :tw],
                            w2_sb[:, ub, db],
                            vt[:, db, off : off + tw],
                            start=(db == 0),
                            stop=(db == NDB - 1),
                        )
                    th = thpool.tile([P, TTILE], mmdt, tag="th")
                    nc.scalar.activation(
                        out=th[:, :tw],
                        in_=pv[:, :tw],
                        func=AF.Tanh,
                        bias=qb_sb[:, ub, b : b + 1],
                    )
                    # fold Wv on VectorE; partition-sum later via one matmul
                    if last_tile:
                        nc.tensor.matmul(
                            ps[:, :tw],
                            wv_col(ub),
                            th[:, :tw],
                            start=(ub == 0),
                            stop=(ub == NUB - 1),
                        )
                    elif ub == 0:
                        nc.vector.tensor_scalar_mul(
                            out=acc_th[:, :tw],
                            in0=th[:, :tw],
                            scalar1=wv32[:, ub : ub + 1],
                        )
                    else:
                        th2 = thpool.tile([P, TTILE], mmdt, tag="th2")
                        nc.vector.tensor_scalar_mul(
                            out=th2[:, :tw],
                            in0=th[:, :tw],
                            scalar1=wv32[:, ub : ub + 1],
                        )
                        nc.vector.tensor_add(
                            acc_th[:, :tw], acc_th[:, :tw], th2[:, :tw]
                        )
                if not last_tile:
                    nc.tensor.matmul(
                        ps[:, :tw],
                        smalls[:, SM_ONE : SM_ONE + 1],
                        acc_th[:, :tw],
                        start=True,
                        stop=True,
                    )
                # exp(score) with fused partial-sum for Z (softmax needs no
                # max-subtraction: |score| <= sum|Wv| ~ 26, safe in fp32)
                nc.scalar.activation(
                    out=e_sb[:, tsl],
                    in_=ps[:, :tw],
                    func=AF.Exp,
                    accum_out=z_sb[:, ti : ti + 1],
                )
                # broadcast e across partitions via K=1 ones-matmul (f32r),
                # bounce to SBUF on ScalarE so the muls run in DVE fast mode
                pb = ppb.tile([P, TTILE], f32, tag="pb")
                nc.tensor.matmul(
                    pb[:, :tw], ones_row, e_sb[:, tsl], start=True, stop=True
                )
                pb_sb = thpool.tile([P, TTILE], mmdt, tag="pb_sb")
                nc.scalar.copy(pb_sb[:, :tw], pb[:, :tw])
                # ctx_acc[p, db] += sum_t vt[p, db, t] * e[t]
                cols = thpool.tile([P, NDB], f32, tag="cols")
                for db in range(NDB):
                    scr = thpool.tile([P, TTILE], mmdt, tag="scr")
                    nc.vector.tensor_mul(
                        scr[:, :tw], as_dve(vt[:, db, off : off + tw]), pb_sb[:, :tw]
                    )
                    nc.vector.reduce_sum(
                        out=cols[:, db : db + 1],
                        in_=scr[:, :tw],
                        axis=mybir.AxisListType.X,
                    )
                if ti == 0:
                    nc.vector.tensor_copy(ctx_acc, cols)
                else:
                    nc.vector.tensor_add(ctx_acc, ctx_acc, cols)
            # ---- per-batch epilogue: normalize ----
            zsum = opool.tile([1, 1], f32, tag="zsum")
            nc.vector.reduce_sum(
                out=zsum, in_=z_sb[:, : len(tiles)], axis=mybir.AxisListType.X
            )
            rz = opool.tile([1, 1], f32, tag="rz")
            nc.vector.reciprocal(out=rz, in_=zsum)
            aw = opool.tile([1, T], f32, tag="aw")
            nc.scalar.activation(
                out=aw, in_=e_sb.bitcast(f32), func=AF.Copy, scale=rz
            )
            nc.sync.dma_start(out=out_attn[b : b + 1, :], in_=aw)
            prz = ppb.tile([P, 1], f32, tag="pb")
            nc.tensor.matmul(prz, ones_row.bitcast(f32), rz, start=True, stop=True)
            rz128 = opool.tile([P, 1], f32, tag="rz128")
            nc.vector.tensor_copy(rz128, prz)
            ctxo = opool.tile([P, NDB], f32, tag="ctxo")
            nc.scalar.activation(
                out=ctxo, in_=ctx_acc, func=AF.Copy, scale=rz128
            )
            nc.sync.dma_start(out=out_ctx[b], in_=ctxo)

    nc.compile()
    return nc


def _np_dt(mm_dt_name):
    if mm_dt_name == "bfloat16":
        import ml_dtypes

        return np.dtype(ml_dtypes.bfloat16)
    return np.float32


def make_in_maps(query, values, W1, b1, W2, b2, Wv, bv, mm_dt_name=MM_DT):
    """Shard + pre-transpose host-side inputs for the 8 cores."""
    del bv  # shift-invariant under softmax; cancels in both outputs
    ndt = _np_dt(mm_dt_name)
    q = np.asarray(query, np.float32)
    # values -> SBUF image [B, NTT, P, NDB, TTILE]: vimg[b,tt,p,db,t] =
    # values[b, tt*TTILE+t, db*P+p]
    vimg = np.ascontiguousarray(
        np.asarray(values, np.float32)
        .reshape(B, NTT, TTILE, NDB, P)
        .transpose(0, 1, 4, 3, 2)
    ).astype(ndt)
    # weights -> SBUF image [P, NUB, NDB, P]: img[p,ub,db,u] = W[db*P+p, ub*P+u]
    def wimg(W):
        return np.ascontiguousarray(
            np.asarray(W, np.float32)
            .reshape(NDB, P, NUB, P)
            .transpose(1, 2, 0, 3)
        ).astype(ndt)

    W1c, W2c = wimg(W1), wimg(W2)
    in_maps = []
    for i in range(N_CORES):
        s = slice(i * BPC, (i + 1) * BPC)
        smalls = np.zeros((P, 64), np.float32)
        smalls[:, SM_ONE] = 1.0
        smalls[:, SM_B1 : SM_B1 + 8] = np.asarray(b1, np.float32).reshape(8, P).T
        smalls[:, SM_B2 : SM_B2 + 8] = np.asarray(b2, np.float32).reshape(8, P).T
        smalls[:, SM_WV : SM_WV + 8] = (
            np.asarray(Wv, np.float32).reshape(8, P, 1)[:, :, 0].T
        )
        # query^T packed: smalls[p, SM_QT + db*4 + b] = query[s][b, db*128+p]
        qs = q[s]  # [4, 1024]
        smalls[:, SM_QT : SM_QT + 32] = (
            qs.reshape(BPC, NDB, P).transpose(2, 1, 0).reshape(P, NDB * BPC)
        )
        in_maps.append(
            {
                "vimg": vimg[s],
                "W1": W1c,
                "W2": W2c,
                "smalls": smalls.astype(ndt),
                "wv32": np.ascontiguousarray(
                    np.asarray(Wv, np.float32).reshape(8, P, 1)[:, :, 0].T
                ),
                "ones": np.ones((1, P), np.float32),
            }
        )
    return in_maps


def kernel(query, values, W1, b1, W2, b2, Wv, bv):
    from concourse.bass_utils import run_bass_kernel_spmd

    if "nc" not in _cache:
        _cache["nc"] = build_nc()
    nc = _cache["nc"]
    in_maps = make_in_maps(query, values, W1, b1, W2, b2, Wv, bv)
    res = run_bass_kernel_spmd(nc, in_maps, core_ids=list(range(N_CORES)))
    ctx = np.concatenate(
        [
            res.results[i]["out_ctx"].transpose(0, 2, 1).reshape(BPC, D)
            for i in range(N_CORES)
        ],
        axis=0,
    )
    aw = np.concatenate([res.results[i]["out_attn"] for i in range(N_CORES)], axis=0)
    return ctx, aw[:, :, None]
